# revision 42
# baseline (speedup 1.0000x reference)
"""Fused multi-head attention (B=4, S=2048, D=1024, H=16, Dh=64, RoPE) on 8 NeuronCores.

Sharding: core = (batch b, head-group g) with b = core//2, g = core%2.
Each core computes its batch's 8 heads end-to-end; host sums the two Wout
row-slice partials per batch.

The schedule is tuned against a dual constraint: the PE sequencer dispatches
every matmul in ~113ns (2-byte moving operands add an Ldweights) while the
engine costs out_free_size cycles. Stage C (attn @ v) runs in a HYBRID of two
forms balancing dispatch-bound vs engine-bound work:
  c_new: out[i, d] with et stationary - 128 matmuls of N=65 per i-block;
     engine-cheap (full 128-partition output), dispatch-heavy. The ones
     column of v_aug lands the softmax denominator in column 64; normalize =
     DVE reciprocal + tensor_scalar mul; the [i, f] result goes through the
     XBAR DMA transpose (SBUF->SBUF) into outT_sb - no PE/PSUM involved.
  c_old (i-blocks in OLD_C): out[d+1, i] with v_aug stationary - 32 matmuls
     of N=512; dispatch-cheap, engine-heavy (65/128 partition occupancy);
     normalize = reciprocal + gpsimd partition_broadcast + DVE mul.
All-fp16 dataflow (host casts inputs; psum accumulates in f32): x tiles are
resident in SBUF (loaded once, startup DMAs in consumption order). RoPE =
two DVE muls straight out of PSUM, four partition-SHIFTED single-input
copies (rotate-half; two-SBUF-input ops may not cross partition bases, but
single-input copies may), and one aligned add; the sign is folded into the
host sin table. exp on ACT (scale fused, no max-subtraction: |sim|*scale is
O(6)) writes fp16 et tiles (44 rotating buffers ~ a 2.5-quantum C deadline).
Emission interleaves everything into the attention j-loop via a TWO-LEVEL
drip queue of generator work items (2 steps per j): stage-A groups of the
next pair jump the main queue at (t,3) (never past a half-emitted head -
that reorders psum-buffer claims vs emission order and deadlocks the
in-order engine queues); the v-projection is split by head-half, with the
half pair-0/1 do not read routed through a background queue that drains
only when the main queue is empty - soaking up mid-run PE slack instead of
starving the exp engine during pair-0's front-load (force-drained before
pair-2's C, its first consumer, is emitted); D groups drip through pair 3.
~327us: PE-engine ~281us busy, ACT (exp chain) ~267us busy.
"""
import sys

for p in ("/opt/trn_rl_repo",):
    if p not in sys.path:
        sys.path.insert(0, p)

from collections import deque

import numpy as np

import concourse.bacc as bacc
import concourse.bass as bass
import concourse.tile as tile
from concourse import mybir
from concourse.bass_utils import run_bass_kernel_spmd

P = 128
S = 2048
D = 1024
NH = 8            # heads per core
DH = 64
SB = 512          # matmul free-dim block
NSB = S // SB     # 4 s-blocks
KD = D // P       # 8 contraction tiles over d
ST = S // P       # 16 s partition-tiles (keys)
FV = NH * DH      # 512 features for this head group
NPAIR = NH // 2   # 4 head pairs
N_CORES = 8
SCALE = DH ** -0.5

f32 = mybir.dt.float32
f32r = mybir.dt.float32r
f16 = mybir.dt.float16

# global i-block indices (4*t + ib) computed with the dispatch-cheap old form;
# placed in the ACT-bound middle region where the PE has engine slack.
# g=15 is always computed inline (old form, streamed) in the last quantum.
OLD_C = {5, 10}


def build_program(old_c=OLD_C, drip_per_j=2, inline_tail=False, a_first=True,
                  a_at_ib=3, v_delay=False, wv_early=True, defer_c0=True,
                  d_pool_evac=False, drip_pair3=None, tail_weave=False,
                  q_drip=False, v1_at=(0, 2), d_bg=False, v_quarters=False):
    if drip_pair3 is None:
        drip_pair3 = drip_per_j
    nc = bacc.Bacc("TRN2", target_bir_lowering=False, debug=False,
                   enable_asserts=False, num_devices=N_CORES)

    xT = nc.dram_tensor("xT", [D, S], f16, kind="ExternalInput").ap()
    # wqk layout (host-packed, pair-major): cols [pair][q 128 | k 128]
    wqk = nc.dram_tensor("wqk", [D, 2 * FV], f16, kind="ExternalInput").ap()
    wv = nc.dram_tensor("wv", [D, FV], f16, kind="ExternalInput").ap()
    wout = nc.dram_tensor("wout", [FV, D], f16, kind="ExternalInput").ap()
    cosb = nc.dram_tensor("cosb", [P, S], f16, kind="ExternalInput").ap()
    sinb = nc.dram_tensor("sinb", [P, S], f16, kind="ExternalInput").ap()
    outT = nc.dram_tensor("outT", [D, S], f16, kind="ExternalOutput").ap()

    with tile.TileContext(nc) as tc:
        with tc.tile_pool(name="persist", bufs=1) as pp, \
             tc.tile_pool(name="psum", bufs=1, space="PSUM") as psp, \
             tc.tile_pool(name="xw", bufs=1) as xw, \
             tc.tile_pool(name="rope", bufs=1) as rp, \
             tc.tile_pool(name="etp", bufs=1) as ep, \
             tc.tile_pool(name="misc", bufs=1) as mp:

            # ---- persistent SBUF ----
            v_sb = [pp.tile([P, NH * (DH + 1)], f16, tag=f"v{i}", name=f"v{i}")
                    for i in range(ST)]
            outT_sb = [[pp.tile([P, SB], f16, tag=f"ot{t}_{ib}", name=f"ot{t}_{ib}")
                        for ib in range(NSB)] for t in range(NPAIR)]
            ones8 = pp.tile([P, NH], f16, tag="ones8", name="ones8")
            nc.vector.memset(ones8[:], 1.0)
            for i in range(ST):
                od = v_sb[i].rearrange("p (h e) -> p h e", h=NH)[:, :, DH]
                nc.vector.tensor_copy(od, ones8[:])

            # x resident: 32 tiles [P, SB] fp16, loaded once
            xts = [[xw.tile([P, SB], f16, tag=f"x{nb}_{k}", name=f"x{nb}_{k}")
                    for k in range(KD)] for nb in range(NSB)]

            def load_wsl(t):
                ws = [xw.tile([P, 2 * P], f16, tag=f"wsl{k}", bufs=2,
                              name=f"wsl{k}") for k in range(KD)]
                for k in range(KD):
                    nc.sync.dma_start(ws[k][:],
                                      wqk[P * k:P * (k + 1), 2 * P * t:2 * P * (t + 1)])
                return ws

            # startup-ordered loads: pair0 weights + x nb0 interleaved first
            wsl = {}
            wsl0 = []
            for k in range(KD):
                w0 = xw.tile([P, 2 * P], f16, tag=f"wsl{k}", bufs=2, name=f"wsl{k}")
                nc.sync.dma_start(w0[:], wqk[P * k:P * (k + 1), 0:2 * P])
                wsl0.append(w0)
                nc.sync.dma_start(xts[0][k][:], xT[P * k:P * (k + 1), 0:SB])
            wsl[0] = wsl0

            cos_sb = mp.tile([P, S], f16, tag="cos", name="cos_sb")
            sin_sb = mp.tile([P, S], f16, tag="sin", name="sin_sb")
            nc.sync.dma_start(cos_sb[:], cosb[:, :])
            nc.sync.dma_start(sin_sb[:], sinb[:, :])
            wv_sb = [xw.tile([P, FV], f16, tag=f"wv{k}", name=f"wv{k}")
                     for k in range(KD)]
            order = ([("wv", k) for k in range(KD)] + [("x1", k) for k in range(KD)])                 if wv_early else                 ([("x1", k) for k in range(KD)] + [("wv", k) for k in range(KD)])
            for kind, k in order:
                if kind == "wv":
                    nc.sync.dma_start(wv_sb[k][:], wv[P * k:P * (k + 1), :])
                else:
                    nc.sync.dma_start(xts[1][k][:], xT[P * k:P * (k + 1), SB:2 * SB])
            for nb in range(2, NSB):
                for k in range(KD):
                    nc.sync.dma_start(xts[nb][k][:],
                                      xT[P * k:P * (k + 1), SB * nb:SB * (nb + 1)])

            wout_sb = []

            def load_wout():
                # wout reuses the dead wv SBUF slots: tile (k, half) holds
                # wout[128k:+128, 512*half:+512]
                for k in range(FV // P):
                    for half in range(2):
                        w = xw.tile([P, FV], f16, tag=f"wv{2 * k + half}", name="wo")
                        nc.sync.dma_start(
                            w[:], wout[P * k:P * (k + 1), FV * half:FV * (half + 1)])
                        wout_sb.append(w)
                yield

            # ---- psum ----
            def sim_tile():
                return psp.tile([P, 2 * SB], f32, tag="sim", bufs=2, name="sim")

            def work_tile():
                return psp.tile([P, SB], f32, tag="work", bufs=4, name="work")

            # ---- stage A (generator; rope fused: psn is per-nb scratch) ----
            def a_group(t, qk, nb, r):
                sl = slice(SB * nb, SB * (nb + 1))
                ps = work_tile()
                for k in range(KD):
                    nc.tensor.matmul(ps[:], wsl[t][k][:, P * qk:P * (qk + 1)],
                                     xts[nb][k][:],
                                     start=(k == 0), stop=(k == KD - 1))
                    if k % 2 == 1 and k < KD - 1:
                        yield
                psn = rp.tile([P, SB], f16, tag="psn", bufs=2, name="psn")
                psw = rp.tile([P, SB], f16, tag="psw", bufs=2, name="psw")
                nc.vector.tensor_mul(r[:, sl], ps[:], cos_sb[:, sl])
                nc.vector.tensor_mul(psn[:], ps[:], sin_sb[:, sl])
                # rotate-half: partition-shifted copies (single-input ops may
                # cross partition bases; two-SBUF-input ops may not)
                for blk in range(4):
                    a, b = 32 * blk, 32 * (blk ^ 1)
                    nc.vector.tensor_copy(psw[a:a + 32, :], psn[b:b + 32, :])
                nc.vector.tensor_add(r[:, sl], r[:, sl], psw[:])
                yield

            # ---- stage A: v projection (generator: 4 steps) ----
            # split by head-half: half 0 (pairs 0-1) is needed by the first
            # C blocks; half 1 (pairs 2-3) defers into pair-1's window
            def v_group(nb, st, half):
                hsl = slice(2 * FV // 4 * half, 2 * FV // 4 * (half + 1))
                psv = work_tile()
                for k in range(KD):
                    nc.tensor.matmul(psv[:, 0:FV // 2],
                                     xts[nb][k][:, P * st:P * (st + 1)],
                                     wv_sb[k][:, hsl],
                                     start=(k == 0), stop=(k == KD - 1))
                    if k % 2 == 1 and k < KD - 1:
                        yield
                s_idx = NSB * nb + st
                v3 = v_sb[s_idx].rearrange("p (h e) -> p h e", h=NH)
                vdst = v3[:, 4 * half:4 * (half + 1), 0:DH]
                vsrc = psv[:, 0:FV // 2].rearrange("p (h e) -> p h e", h=NH // 2)
                nc.vector.tensor_copy(vdst, vsrc)
                yield

            def v_quarter(nb, st, qtr):
                qsl = slice(P * qtr, P * (qtr + 1))
                psv = work_tile()
                for k in range(KD):
                    nc.tensor.matmul(psv[:, 0:P],
                                     xts[nb][k][:, P * st:P * (st + 1)],
                                     wv_sb[k][:, qsl],
                                     start=(k == 0), stop=(k == KD - 1))
                    if k % 2 == 1 and k < KD - 1:
                        yield
                s_idx = NSB * nb + st
                v3 = v_sb[s_idx].rearrange("p (h e) -> p h e", h=NH)
                vdst = v3[:, 2 * qtr:2 * (qtr + 1), 0:DH]
                vsrc = psv[:, 0:P].rearrange("p (h e) -> p h e", h=2)
                nc.vector.tensor_copy(vdst, vsrc)
                yield

            # ---- stage B + exp ----
            def b_exp(rq, rk, ib, ets, j):
                isl = slice(SB * ib, SB * (ib + 1))
                sim = sim_tile()
                for hh in range(2):
                    off = DH * hh
                    nc.tensor.matmul(sim[:, SB * hh:SB * (hh + 1)],
                                     rk[off:off + DH, P * j:P * (j + 1)],
                                     rq[off:off + DH, isl],
                                     start=True, stop=True,
                                     tile_position=(off, 0))
                et = ep.tile([P, 2 * SB], f16, tag="et", bufs=44, name="et")
                nc.scalar.activation(et[:], sim[:],
                                     mybir.ActivationFunctionType.Exp,
                                     scale=SCALE)
                ets[j] = et

            # ---- stage C, new form: [i, d] with et stationary ----
            def c_new(t, ib, ets):
                for c in range(NSB):
                    tr = mp.tile([P, P], f16, tag="tr", bufs=3, name="tr")
                    for hh in range(2):
                        w = work_tile()
                        h = 2 * t + hh
                        for j in range(ST):
                            nc.tensor.matmul(
                                w[:, 0:DH + 1],
                                ets[j][:, SB * hh + P * c:SB * hh + P * (c + 1)],
                                v_sb[j][:, (DH + 1) * h:(DH + 1) * h + DH + 1],
                                start=(j == 0), stop=(j == ST - 1))
                            if j % 4 == 3 and j < ST - 1:
                                yield
                        rec = mp.tile([P, 1], f32, tag="rec", bufs=4, name="rec")
                        nc.vector.reciprocal(rec[:], w[:, DH:DH + 1])
                        nc.vector.tensor_scalar_mul(tr[:, DH * hh:DH * (hh + 1)],
                                                    w[:, 0:DH], rec[:])
                        yield
                    nc.sync.dma_start_transpose(
                        outT_sb[t][ib][:, P * c:P * (c + 1)], tr[:])

            # ---- stage C, old form: [d+1, i] with v_aug stationary ----
            def c_old(t, ib, ets):
                for hh in range(2):
                    w = work_tile()
                    h = 2 * t + hh
                    for j in range(ST):
                        nc.tensor.matmul(w[0:DH + 1, :],
                                         v_sb[j][:, (DH + 1) * h:(DH + 1) * h + DH + 1],
                                         ets[j][:, SB * hh:SB * (hh + 1)],
                                         start=(j == 0), stop=(j == ST - 1))
                        if j % 2 == 1 and j < ST - 1:
                            yield
                    rrow = mp.tile([1, SB], f16, tag="rrow", bufs=2, name="rrow")
                    with nc.allow_low_precision(reason="softmax denom recip in fp16"):
                        nc.vector.reciprocal(rrow[0:1, :], w[DH:DH + 1, :])
                    bc = mp.tile([DH, SB], f16, tag="bc", bufs=2, name="bc")
                    nc.gpsimd.partition_broadcast(bc[:], rrow[0:1, :])
                    nc.vector.tensor_mul(outT_sb[t][ib][DH * hh:DH * (hh + 1), :],
                                         w[0:DH, :], bc[:])
                    yield

            # ---- stage D (generator: 2 steps) ----
            def d_group(mi, ib):
                pd = work_tile()
                for k in range(FV // P):
                    wt = wout_sb[2 * k + mi // 4]
                    nc.tensor.matmul(pd[:], wt[:, P * (mi % 4):P * (mi % 4 + 1)],
                                     outT_sb[k][ib][:],
                                     start=(k == 0), stop=(k == FV // P - 1))
                    if k == 1:
                        yield
                ot = mp.tile([P, SB], f16, tag="dout", bufs=3, name="dout")
                if d_pool_evac and mi % 2 == 1:
                    nc.gpsimd.tensor_copy(ot[:], pd[:])
                else:
                    nc.vector.tensor_copy(ot[:], pd[:])
                nc.sync.dma_start(outT[P * mi:P * (mi + 1), SB * ib:SB * (ib + 1)],
                                  ot[:])
                yield

            # ---- drip scheduler: two-level queue of generators ----
            # dq_bg drains only when dq is empty (soaks up PE slack without
            # displacing deadline-critical C/A work); claims stay in emission
            # order so the tile-pool rotation can never create a cycle
            dq = deque()
            dq_bg = deque()
            started = set()

            def drip(n=drip_per_j):
                done = 0
                while done < n and (dq or dq_bg):
                    q = dq if dq else dq_bg
                    started.add(id(q[0]))
                    try:
                        next(q[0])
                        done += 1
                    except StopIteration:
                        q.popleft()

            def push_front(gens):
                # never jump ahead of a half-emitted generator (it may hold a
                # psum buffer whose release depends on its later instructions)
                pos = 1 if dq and id(dq[0]) in started else 0
                for g in reversed(gens):
                    dq.insert(pos, g)

            def c_form(t, ib, ets):
                return c_old(t, ib, ets) if (NSB * t + ib) in old_c \
                    else c_new(t, ib, ets)

            # ---- main schedule ----
            def r_tiles():
                rq = rp.tile([P, S], f16, tag="rq", bufs=2, name="rq")
                rk = rp.tile([P, S], f16, tag="rk", bufs=2, name="rk")
                return rq, rk

            rq0, rk0 = r_tiles()
            ets_cur = {}

            for nb in range(NSB):
                if nb == 0:
                    for _ in a_group(0, 0, nb, rq0):
                        pass
                elif q_drip:
                    dq.append(a_group(0, 0, nb, rq0))
                else:
                    for _ in a_group(0, 0, nb, rq0):
                        pass
                if not v_delay:
                    for st in range(NSB):
                        if v_quarters:
                            dq.append(v_quarter(nb, st, 0))
                            dq_bg.append(v_quarter(nb, st, 1))
                        else:
                            dq.append(v_group(nb, st, 0))
                for _ in a_group(0, 1, nb, rk0):
                    pass
                for j in range(NSB * nb, NSB * nb + NSB):
                    b_exp(rq0, rk0, 0, ets_cur, j)
                    drip()
                if v_delay:
                    for st in range(NSB):
                        dq.append(v_group(nb, st, 0))

            r_pair = {0: (rq0, rk0)}
            quanta = [(t, ib) for t in range(NPAIR) for ib in range(NSB)][1:]

            c_backlog = []
            for t, ib in quanta:
                pt, pib = (t, ib - 1) if ib else (t - 1, NSB - 1)
                ets_prev, ets_cur = ets_cur, {}
                if defer_c0 and pt == 0:
                    c_backlog.append((pt, pib, ets_prev))
                    if len(c_backlog) > 1:
                        dq.append(c_form(*c_backlog.pop(0)))
                else:
                    while c_backlog:
                        dq.append(c_form(*c_backlog.pop(0)))
                    dq.append(c_form(pt, pib, ets_prev))
                if (t, ib) == v1_at:
                    for nb2 in range(NSB):
                        for st in range(NSB):
                            dq_bg.append(v_group(nb2, st, 1))
                if v_quarters and (t, ib) == (1, 1):
                    while len(dq_bg) > NSB * NSB:
                        try:
                            next(dq_bg[0])
                        except StopIteration:
                            dq_bg.popleft()
                if t == 2 and ib == 0:
                    # force-drain: pair-2's C reads half-1 v; its matmuls must
                    # never precede pending v writes in the PE stream
                    while dq_bg:
                        try:
                            next(dq_bg[0])
                        except StopIteration:
                            dq_bg.popleft()
                if ib == a_at_ib and t + 1 < NPAIR:
                    wsl[t + 1] = load_wsl(t + 1)
                    rq_n, rk_n = r_tiles()
                    r_pair[t + 1] = (rq_n, rk_n)
                    if a_first:
                        push_front([g for nb in range(NSB)
                                    for g in (a_group(t + 1, 0, nb, rq_n),
                                              a_group(t + 1, 1, nb, rk_n))])
                    else:
                        for nb in range(NSB):
                            dq.append(a_group(t + 1, 0, nb, rq_n))
                            dq.append(a_group(t + 1, 1, nb, rk_n))
                if t == 2 and ib == 3:
                    dq.append(load_wout())
                if t == 3 and ib >= 1:
                    for mi in range(D // P):
                        (dq_bg if d_bg else dq).append(d_group(mi, ib - 1))

                rq, rk = r_pair[t]
                if inline_tail and (t, ib) == (NPAIR - 1, NSB - 1):
                    # last quantum: stream C inline (old form, j-wise right
                    # behind each exp) so the tail is just normalize + D
                    cw = [work_tile(), work_tile()]
                    for j in range(ST):
                        b_exp(rq, rk, ib, ets_cur, j)
                        for hh in range(2):
                            h = 2 * t + hh
                            nc.tensor.matmul(
                                cw[hh][0:DH + 1, :],
                                v_sb[j][:, (DH + 1) * h:(DH + 1) * h + DH + 1],
                                ets_cur[j][:, SB * hh:SB * (hh + 1)],
                                start=(j == 0), stop=(j == ST - 1))
                        drip(2)
                    for hh in range(2):
                        rrow = mp.tile([1, SB], f32, tag="rrow", bufs=2, name="rrow")
                        nc.vector.reciprocal(rrow[0:1, :], cw[hh][DH:DH + 1, :])
                        bc = mp.tile([DH, SB], f32, tag="bc", bufs=2, name="bc")
                        nc.gpsimd.partition_broadcast(bc[:], rrow[0:1, :])
                        nc.vector.tensor_mul(
                            outT_sb[t][ib][DH * hh:DH * (hh + 1), :],
                            cw[hh][0:DH, :], bc[:])
                else:
                    n = drip_pair3 if t == NPAIR - 1 else drip_per_j
                    for j in range(ST):
                        b_exp(rq, rk, ib, ets_cur, j)
                        drip(n)

            if inline_tail:
                for mi in range(D // P):
                    dq.append(d_group(mi, 3))
                while dq:
                    drip(16)
            elif not tail_weave:
                dq.append(c_form(3, 3, ets_cur))
                for mi in range(D // P):
                    dq.append(d_group(mi, 3))
                while dq:
                    drip(16)
            else:
                # weave D(mi,3) k0..2 prefixes (psum borrowed from the dead
                # sim tiles) between C(3,3) groups so the PE stays busy while
                # the transpose DMAs land; k=3 suffixes close each group
                while dq:
                    drip(16)
                pds = {}

                def d_pre(mi):
                    pd = sim_tile()
                    for k in range(3):
                        wt = wout_sb[2 * k + mi // 4]
                        nc.tensor.matmul(pd[:, 0:SB],
                                         wt[:, P * (mi % 4):P * (mi % 4 + 1)],
                                         outT_sb[k][3][:],
                                         start=(k == 0), stop=False)
                    pds[mi] = pd

                def d_fin(mi):
                    pd = pds.pop(mi)
                    wt = wout_sb[6 + mi // 4]
                    nc.tensor.matmul(pd[:, 0:SB],
                                     wt[:, P * (mi % 4):P * (mi % 4 + 1)],
                                     outT_sb[3][3][:], start=False, stop=True)
                    ot = mp.tile([P, SB], f16, tag="dout", bufs=3, name="dout")
                    nc.vector.tensor_copy(ot[:], pd[:, 0:SB])
                    nc.sync.dma_start(outT[P * mi:P * (mi + 1), 3 * SB:4 * SB],
                                      ot[:])

                cg = c_form(3, 3, ets_cur)
                steps = list(range(8))
                seq = ["c"] * 8 + ["p", 0] + ["c"] * 8 + ["p", 1] + ["c"] * 16
                i = 0
                while i < len(seq):
                    s = seq[i]
                    if s == "c":
                        try:
                            next(cg)
                        except StopIteration:
                            pass
                        i += 1
                    else:
                        d_pre(seq[i + 1])
                        i += 2
                for _ in cg:
                    pass
                for mi in range(D // P):
                    if mi >= 2:
                        d_pre(mi)
                    d_fin(mi if mi < 2 else mi)
                # note: d_pre(mi) for mi>=2 reuses the sim buffer freed by the
                # previous d_fin's evacuation (pool rotation handles the dep)

    nc.compile()
    return nc


_PROG = None


def _get_prog():
    global _PROG
    if _PROG is None:
        _PROG = build_program()
    return _PROG


def make_in_maps(x, Wqkv, Wout):
    BASE = 10000.0
    f = np.arange(32, dtype=np.float64)
    invfreq = BASE ** (-2.0 * f / DH)                      # [32]
    tpos = np.arange(S, dtype=np.float64)
    ang = np.outer(invfreq, tpos)                          # [32, S]
    cos32 = np.cos(ang)
    sin32 = np.sin(ang)
    cosb = np.tile(cos32, (4, 1)).astype(np.float16)       # [128, S]
    # sign indexed by SOURCE row r: -sin when r%64 >= 32
    sgn = np.repeat(np.array([1.0, -1.0, 1.0, -1.0]), 32)[:, None]
    sinb = (np.tile(sin32, (4, 1)) * sgn).astype(np.float16)

    in_maps = []
    for cid in range(N_CORES):
        b, g = divmod(cid, 2)
        xTc = np.ascontiguousarray(x[b].T).astype(np.float16)  # [D, S]
        qcols = []
        for t in range(NPAIR):
            qcols.append(Wqkv[:, 512 * g + 128 * t:512 * g + 128 * (t + 1)])
            qcols.append(Wqkv[:, 1024 + 512 * g + 128 * t:1024 + 512 * g + 128 * (t + 1)])
        wqk_c = np.ascontiguousarray(np.concatenate(qcols, axis=1)).astype(np.float16)
        wv_c = np.ascontiguousarray(
            Wqkv[:, 2048 + 512 * g:2048 + 512 * (g + 1)]).astype(np.float16)
        wout_c = np.ascontiguousarray(Wout[512 * g:512 * (g + 1), :]).astype(np.float16)
        in_maps.append({"xT": xTc, "wqk": wqk_c, "wv": wv_c, "wout": wout_c,
                        "cosb": cosb, "sinb": sinb})
    return in_maps


def gather_output(results, B=4):
    outs = []
    for b in range(B):
        acc = results[2 * b]["outT"].astype(np.float32) \
            + results[2 * b + 1]["outT"].astype(np.float32)
        outs.append(acc.T)
    return np.stack(outs, axis=0)


def kernel(x, Wqkv, Wout):
    x = np.asarray(x, dtype=np.float32)
    Wqkv = np.asarray(Wqkv, dtype=np.float32)
    Wout = np.asarray(Wout, dtype=np.float32)
    nc = _get_prog()
    in_maps = make_in_maps(x, Wqkv, Wout)
    res = run_bass_kernel_spmd(nc, in_maps, core_ids=list(range(N_CORES)))
    return gather_output(res.results, B=x.shape[0])


if __name__ == "__main__":
    rng = np.random.default_rng(0)
    x = rng.standard_normal((4, S, D)).astype(np.float32)
    Wqkv = (rng.standard_normal((D, 3 * D)) * D ** -0.5).astype(np.float32)
    Wout = (rng.standard_normal((D, D)) * D ** -0.5).astype(np.float32)
    out = kernel(x, Wqkv, Wout)
    print("kernel ran, out shape:", out.shape, "finite:", np.isfinite(out).all())
